# revision 1
# baseline (speedup 1.0000x reference)
"""Trainium2 Bass kernel for ToRA-adapted windowed attention block.

Math (per batch image, S=1024 tokens, dim=768, 12 heads x 64):
  qkv  = x @ (Wqkv + U1 Gt U2^T)^T + b          Gt = G . U3[task]
  q,k,v split; attn = softmax(q k^T / 8) v ; out = attn-merge
  y    = out @ (Wp + U1p Gtp U2p^T)^T + bp

Strategy: data-parallel over B=8 — one image per NeuronCore, no
collectives. Device pipeline is feature-major:
  - host pre-computes effective weights (tiny low-rank update) and
    pre-transposes weights + x so the contract dim lands on SBUF
    partitions.
  - QKV^T computed feature-major for Q,K (gives Q^T/K^T tiles directly);
    V computed token-major and augmented with a ones-column so the
    A^T @ V_aug matmul also yields softmax denominators for free.
  - scores are computed twice on PE (cheap with f32r @ 1 cyc/row):
    once q-major subsampled (stride 4) for row-max stats, once k-major
    with an appended (-max - margin) row folded into the contraction, so
    exp() needs no per-column bias and A^T comes out k-major, ready to
    contract with V.
  - proj consumes attention output feature-major; softmax 1/l scaling is
    applied during the attention evacuation via a PE rank-1 broadcast.
All big matmuls run as float32r (FP22 multiply, FP32 accumulate).
"""

import os
import sys
import numpy as np

sys.path.insert(0, "/opt/trn_rl_repo")

import concourse.bass as bass
import concourse.tile as tile
from concourse import bacc, mybir
from concourse.bass_utils import run_bass_kernel_spmd
from concourse.masks import make_identity

F32 = mybir.dt.float32
F32R = mybir.dt.float32r
AX = mybir.AxisListType.X
OP = mybir.AluOpType
EXP = mybir.ActivationFunctionType.Exp
LN = mybir.ActivationFunctionType.Ln
IDENT = mybir.ActivationFunctionType.Identity

D = 768          # model dim
KT = 6           # contract tiles over D
S = 1024         # tokens per image
NH = 12
HD = 64
MARGIN = 1.0     # safety margin over the stats-pass row-max

N_CORES = 8


def build_program():
    import os as _os
    SKIP = set(_os.environ.get("K_SKIP", "").split(","))
    LSE_HEADS = {
        int(v) for v in _os.environ.get("K_LSE", "2,6,10").split(",") if v != ""
    }
    nc = bacc.Bacc(
        "TRN2",
        target_bir_lowering=False,
        debug=False,
        enable_asserts=True,
        num_devices=N_CORES,
    )
    xT = nc.dram_tensor("xT", [D, S], F32, kind="ExternalInput").ap()
    WqkT = nc.dram_tensor("WqkT", [D, 2 * D], F32, kind="ExternalInput").ap()
    WvT = nc.dram_tensor("WvT", [D, D], F32, kind="ExternalInput").ap()
    WpT = nc.dram_tensor("WpT", [D, D], F32, kind="ExternalInput").ap()
    bqk = nc.dram_tensor("bqk", [128, 12], F32, kind="ExternalInput").ap()
    bv = nc.dram_tensor("bv", [D], F32, kind="ExternalInput").ap()
    bp = nc.dram_tensor("bp", [D], F32, kind="ExternalInput").ap()
    y = nc.dram_tensor("y", [S, D], F32, kind="ExternalOutput").ap()

    def bcast128(v):
        return bass.AP(tensor=v.tensor, offset=v.offset, ap=[[0, 128], [1, D]])

    with tile.TileContext(nc) as tc:
        with tc.tile_pool(name="persist", bufs=1) as persist:
            # long-lived tiles
            QKT = persist.tile([128, 12, S], F32R)      # Q^T, K^T feature-major
            Vaug = persist.tile([128, 8, NH * 65], F32R)  # V tok-major + ones col
            attnT = persist.tile([128, KT, S], F32R)      # attn out^T, normalized
            bqk_sb = persist.tile([128, 12], F32)
            bvb = persist.tile([128, D], F32)
            bpb = persist.tile([128, D], F32)
            ident = persist.tile([128, 128], F32)
            identr = persist.tile([128, 128], F32R)
            ones1 = persist.tile([1, 64], F32R)

            nc.sync.dma_start(out=bqk_sb, in_=bqk)
            nc.sync.dma_start(out=bvb, in_=bcast128(bv))
            nc.sync.dma_start(out=bpb, in_=bcast128(bp))
            make_identity(nc, ident)
            nc.vector.tensor_scalar(
                out=identr, in0=ident, scalar1=1.0, scalar2=None, op0=OP.mult
            )
            ONE_F32 = 0x3F800000
            nc.vector.memset(ones1.bitcast(mybir.dt.uint32), ONE_F32)
            nc.vector.memset(Vaug.bitcast(mybir.dt.uint32), ONE_F32)

            # ---------------- stage A: QKV ----------------
            with (
                tc.tile_pool(name="qkvw", bufs=1) as qkvw,
                tc.tile_pool(name="vtmpp", bufs=3) as vtmpp,
                tc.tile_pool(name="qkps", bufs=3, space="PSUM") as qkps,
                tc.tile_pool(name="vps", bufs=2, space="PSUM") as vps,
            ):
                xT_sb = qkvw.tile([128, KT, S], F32R)
                WqkT_sb = qkvw.tile([128, KT, 2 * D], F32R)
                WvT_sb = qkvw.tile([128, KT, D], F32R)
                xT_r = xT.rearrange("(k p) t -> p k t", p=128).bitcast(F32R)
                WqkT_r = WqkT.rearrange("(k p) f -> p k f", p=128).bitcast(F32R)
                WvT_r = WvT.rearrange("(k p) f -> p k f", p=128).bitcast(F32R)
                for kt in range(KT):
                    nc.sync.dma_start(
                        out=WqkT_sb[:, kt, :], in_=WqkT_r[:, kt, :]
                    )
                    nc.sync.dma_start(out=xT_sb[:, kt, :], in_=xT_r[:, kt, :])
                for kt in range(KT):
                    nc.sync.dma_start(out=WvT_sb[:, kt, :], in_=WvT_r[:, kt, :])

                # V token-major: out[tok, feat] -> Vaug slices (+bias)
                for tt in ([] if 'qkv' in SKIP else range(8)):
                    psv = vps.tile([128, D], F32, tag="psv")
                    for kt in range(KT):
                        for f0, fl in ((0, 512), (512, 256)):
                            nc.tensor.matmul(
                                psv[:, f0 : f0 + fl],
                                (xT_sb[:, kt, tt * 128 : (tt + 1) * 128]),
                                (WvT_sb[:, kt, f0 : f0 + fl]),
                                start=(kt == 0),
                                stop=(kt == KT - 1),
                            )
                    vtmp = vtmpp.tile([128, D], F32R, tag="vtmp")
                    nc.vector.tensor_add(vtmp[:, 0:384], psv[:, 0:384], bvb[:, 0:384])
                    nc.vector.tensor_add(
                        vtmp[:, 384:768], psv[:, 384:768], bvb[:, 384:768]
                    )
                    for h in range(NH):
                        nc.gpsimd.tensor_copy(
                            Vaug[:, tt, h * 65 : h * 65 + 64],
                            vtmp[:, h * 64 : (h + 1) * 64],
                        )

                # Q^T / K^T feature-major: out[feat, tok]
                for ft in ([] if 'qkv' in SKIP else [0, 6, 1, 7, 2, 8, 3, 9, 4, 10, 5, 11]):
                    ps_a = qkps.tile([128, 512], F32, tag="qkps")
                    ps_b = qkps.tile([128, 512], F32, tag="qkps")
                    pss2 = [ps_a, ps_b]
                    for kt in range(KT):
                        for qc in range(2):
                            nc.tensor.matmul(
                                pss2[qc],
                                (WqkT_sb[:, kt, ft * 128 : (ft + 1) * 128]),
                                (xT_sb[:, kt, qc * 512 : (qc + 1) * 512]),
                                start=(kt == 0),
                                stop=(kt == KT - 1),
                            )
                    for qc in range(2):
                        dst = QKT[:, ft, qc * 512 : (qc + 1) * 512]
                        # ACT evac: Identity(psum*scale + bias); host pre-scales
                        # the Q bias by 1/8 so (raw+b)/8 == raw*0.125 + b/8
                        nc.scalar.activation(
                            dst, pss2[qc], IDENT,
                            bias=bqk_sb[:, ft : ft + 1],
                            scale=0.125 if ft < 6 else 1.0,
                        )

            # ---------------- stage B: attention ----------------
            with tc.tile_pool(name="late", bufs=1) as late:
              WpT_sb = late.tile([128, KT, D], F32R)
              nc.sync.dma_start(
                  out=WpT_sb, in_=WpT.rearrange("(k p) f -> p k f", p=128).bitcast(F32R)
              )
              with (
                tc.tile_pool(name="qkbuf", bufs=4) as qkbuf,
                tc.tile_pool(name="ysb", bufs=2) as ysb,
                tc.tile_pool(name="onebuf", bufs=1) as onebuf,
                tc.tile_pool(name="atp", bufs=2) as atp,
                tc.tile_pool(name="stat", bufs=2) as statp,
                tc.tile_pool(name="aps1", bufs=1, space="PSUM") as aps1,
                tc.tile_pool(name="apss", bufs=2, space="PSUM") as apss,
                tc.tile_pool(name="aps2", bufs=2, space="PSUM") as aps2,
                tc.tile_pool(name="apso", bufs=2, space="PSUM") as apso,
              ):
                def phase1(h):
                    fq, off = h // 2, (h % 2) * 64
                    fk = 6 + fq
                    # Ktilde = [K^T ; ones], Qtilde = [Q^T/8 ; -(c_q)]
                    Kt = qkbuf.tile([65, S], F32R, tag="Kt")
                    nc.gpsimd.tensor_copy(Kt[0:64, :], QKT[off : off + 64, fk, :])
                    nc.gpsimd.memset(
                        Kt[64:65, :].bitcast(mybir.dt.uint32), 0x3F800000
                    )
                    Qt = qkbuf.tile([65, S], F32R, tag="Qt")
                    nc.gpsimd.tensor_copy(Qt[0:64, :], QKT[off : off + 64, fq, :])

                    use_lse = h in LSE_HEADS
                    mcol = statp.tile([128, 8], F32R, tag="mcol")
                    for qt in range(8):
                        mq = statp.tile([128, 2], F32, tag="mq")
                        for kc in range(2):
                            pss = apss.tile([128, 512], F32, tag="pss")
                            nc.tensor.matmul(
                                pss,
                                QKT[off : off + 64, fq, qt * 128 : (qt + 1) * 128],
                                QKT[off : off + 64, fk, kc * 512 : (kc + 1) * 512],
                                start=True,
                                stop=True,
                            )
                            if use_lse:
                                # ACT-side stats: l0 = sum exp(s/16); c_q
                                # bound = 16*ln(l0) - 40 (safe: slack<=104,
                                # window [-40, +64] around rowmax)
                                esc = statp.tile([128, 512], F32, tag="esc")
                                nc.scalar.activation(
                                    esc, pss, EXP, scale=0.0625,
                                    accum_out=mq[:, kc : kc + 1],
                                )
                            else:
                                nc.vector.reduce_max(
                                    mq[:, kc : kc + 1], pss, axis=AX
                                )
                        nc.vector.tensor_tensor(
                            mcol[:, qt : qt + 1], mq[:, 0:1], mq[:, 1:2],
                            OP.add if use_lse else OP.max,
                        )
                    if use_lse:
                        # bit-hack log2: c = 16*ln2*(bits/2^23 - 127) - 40;
                        # Qt row = -c = -16*ln2/2^23 * bits + (16*127*ln2 + 40)
                        mbits = statp.tile([128, 8], F32R, tag="mbits")
                        nc.vector.tensor_copy(
                            mbits, mcol.bitcast(mybir.dt.int32)
                        )
                        mcol = mbits
                    # row-ify -c: PE transpose, scale/offset on evac, DMA flatten
                    psT = aps1.tile([8, 128], F32R, tag="psT")
                    nc.tensor.transpose(psT, mcol, identr)
                    mstage = statp.tile([8, 128], F32, tag="mstage")
                    if use_lse:
                        nc.vector.tensor_scalar(
                            out=mstage, in0=psT,
                            scalar1=-16.0 * 0.6931471805599453 / 8388608.0,
                            scalar2=16.0 * 127.0 * 0.6931471805599453 + 40.0,
                            op0=OP.mult, op1=OP.add,
                        )
                    else:
                        nc.vector.tensor_scalar(
                            out=mstage, in0=psT, scalar1=-1.0, scalar2=-MARGIN,
                            op0=OP.mult, op1=OP.add,
                        )
                    nc.sync.dma_start(
                        out=Qt[64:65, :].bitcast(F32), in_=mstage
                    )
                    return Kt, Qt

                def phase2(h, Kt, Qt):
                    fq, off = h // 2, (h % 2) * 64
                    # S'^T = Ktilde^T Qtilde (k-major, max pre-subtracted); exp;
                    # then out^T(+denominator row) = Vaug^T A'^T ; normalize
                    for qc in range(2):
                        AT = atp.tile([128, 8, 512], F32R, tag="AT")
                        for kt in range(8):
                            ps2 = aps2.tile([128, 512], F32, tag="ps2")
                            nc.tensor.matmul(
                                ps2,
                                (Kt[:, kt * 128 : (kt + 1) * 128]),
                                (Qt[:, qc * 512 : (qc + 1) * 512]),
                                start=True,
                                stop=True,
                            )
                            nc.scalar.activation(AT[:, kt, :], ps2, EXP)

                        pso = apso.tile([65, 512], F32, tag="pso")
                        for kt in range(8):
                            nc.tensor.matmul(
                                pso,
                                (Vaug[:, kt, h * 65 : (h + 1) * 65]),
                                (AT[:, kt, :]),
                                start=(kt == 0),
                                stop=(kt == 7),
                            )
                        rec = onebuf.tile([1, 512], F32R, tag="rec")
                        with nc.allow_low_precision(reason="softmax 1/l in fp32r"):
                            nc.vector.reciprocal(rec, pso[64:65, :])
                        psb = aps1.tile([64, 512], F32, tag="psb")
                        nc.tensor.matmul(
                            psb, (ones1), (rec), start=True, stop=True
                        )
                        rbc = onebuf.tile([64, 512], F32, tag="rbc")
                        nc.vector.tensor_copy(rbc, psb)
                        nc.vector.tensor_mul(
                            attnT[off : off + 64, fq, qc * 512 : (qc + 1) * 512],
                            pso[0:64, :],
                            rbc,
                        )

                heads = [] if 'attn' in SKIP else list(range(NH))
                PIPE = int(os.environ.get("K_PIPE", "2"))
                pend = []
                for h in heads:
                    if PIPE:
                        pend.append((h, *phase1(h)))
                        if len(pend) > PIPE:
                            phase2(*pend.pop(0))
                    else:
                        phase2(h, *phase1(h))
                for item in pend:
                    phase2(*item)


                # ---------------- proj (interleaved with attention tail) ----
                for tt in ([] if 'proj' in SKIP else range(8)):
                    yt = ysb.tile([128, D], F32, tag="yt")
                    for ci, (f0, fl) in enumerate(((0, 512), (512, 256))):
                        psy = aps2.tile([128, 512], F32, tag="ps2")
                        for kt in range(KT):
                            nc.tensor.matmul(
                                psy[:, 0:fl],
                                (attnT[:, kt, tt * 128 : (tt + 1) * 128]),
                                (WpT_sb[:, kt, f0 : f0 + fl]),
                                start=(kt == 0),
                                stop=(kt == KT - 1),
                            )
                        nc.vector.tensor_add(
                            yt[:, f0 : f0 + fl], psy[:, 0:fl], bpb[:, f0 : f0 + fl]
                        )
                        if ci == 1:
                            nc.sync.dma_start(
                                out=y[tt * 128 : (tt + 1) * 128, :], in_=yt
                            )

    nc.compile()
    return nc


_NC = None


def _get_nc():
    global _NC
    if _NC is None:
        _NC = build_program()
    return _NC


def prep_inputs(x, qkv_w, qkv_b, U1_qkv, U2_qkv, U3_qkv, G_qkv,
                proj_w, proj_b, U1_p, U2_p, U3_p, G_p, task_idx):
    t = int(task_idx)
    f = np.float32
    x = np.asarray(x, f)
    qkv_w = np.asarray(qkv_w, f)
    qkv_b = np.asarray(qkv_b, f)
    proj_w = np.asarray(proj_w, f)
    proj_b = np.asarray(proj_b, f)

    Gt = np.einsum("pqv,v->pq", np.asarray(G_qkv, f), np.asarray(U3_qkv, f)[t])
    Wqkv = qkv_w + np.asarray(U1_qkv, f) @ Gt @ np.asarray(U2_qkv, f).T
    Gtp = np.einsum("pqv,v->pq", np.asarray(G_p, f), np.asarray(U3_p, f)[t])
    Wp = proj_w + np.asarray(U1_p, f) @ Gtp @ np.asarray(U2_p, f).T

    WqkT = np.ascontiguousarray(Wqkv[: 2 * D].T)
    WvT = np.ascontiguousarray(Wqkv[2 * D :].T)
    WpT = np.ascontiguousarray(Wp.T)
    bqk = np.ascontiguousarray(qkv_b[: 2 * D].reshape(12, 128).T)
    bqk[:, 0:6] *= 0.125  # Q bias pre-scaled (ACT evac applies scale to psum only)
    bv = np.ascontiguousarray(qkv_b[2 * D :])
    bp = proj_b

    B = x.shape[0]
    xr = x.reshape(B, S, D)
    in_maps = [
        dict(
            xT=np.ascontiguousarray(xr[c].T),
            WqkT=WqkT, WvT=WvT, WpT=WpT, bqk=bqk, bv=bv, bp=bp,
        )
        for c in range(B)
    ]
    return in_maps


def run(in_maps, trace=False):
    nc = _get_nc()
    res = run_bass_kernel_spmd(nc, in_maps, list(range(N_CORES)), trace=trace)
    return res


def kernel(x, **kw):
    B, H, W, C = x.shape
    in_maps = prep_inputs(x, **kw)
    res = run(in_maps)
    out = np.stack([np.asarray(res.results[c]["y"]) for c in range(B)])
    return out.reshape(B, H, W, C).astype(np.float32)

